# revision 94
# baseline (speedup 1.0000x reference)
"""Causal multi-head attention block (QKV proj -> causal MHA -> out proj) on 8 Trainium2
cores.

Sharding: core = b*2 + hh handles batch b (of 4) and head-half hh (8 of 16 heads),
computing attention for its heads over the full sequence, then a partial output
projection over its 512 y-channels for all 2048 tokens. A pairwise ReduceScatter
([0,1],[2,3],...) sums the two partials of each batch and leaves each core with its
token-half of the final output.

Host-side prep: x^T is laid out as the exact SBUF image (the device transposes
only tokens 0-255, providing cheap PE work that burns the clock-ramp window
while the big DMAs stream in); weights are pre-rearranged to fb-major
partition-major images so the first-needed slices are contiguous full-rate
DMAs; the V bias is folded into the output bias.

Device structure per core:
  - All work is cut into "pieces" (QKV projection halves, x^T transposes,
    y-transposes, out-projection chunks) that are dripped into the attention
    inner loop, which is otherwise Act-bound (softmax exp chain).
  - The attention pipeline is flat across (qtile, head-pair) units: the next
    unit's first scores are emitted inside the previous unit's last key-group
    iteration, so the Act engine never drains at unit boundaries.
  - Causal masking multiplies the diagonal 128x128 bands of the attention
    weights by a 0/1 triangle on the otherwise-idle GPSIMD engine (SBUF only),
    keeping the scores->exp chain free of extra hops.
  - attnV computes y transposed (queries on PSUM partitions, via a ones column
    in V for the softmax denominator) so normalization is a per-partition
    tensor_scalar multiply; y is transposed back on the PE for the out-proj.

dtypes: bf16 matmul operands everywhere; PSUM accumulation f32; the softmax
denominator is accumulated from the same bf16 weights, so normalization is
exact w.r.t. rounding.
"""

import numpy as np

import concourse.bass as bass
import concourse.tile as tile
from concourse import bacc, mybir
from concourse.bass_utils import run_bass_kernel_spmd

F32 = mybir.dt.float32
BF16 = mybir.dt.bfloat16
AF = mybir.ActivationFunctionType

B, T, C, H = 4, 2048, 1024, 16
D = C // H          # 64
NHL = H // 2        # 8 local heads per core
NHP = NHL // 2      # 4 local head pairs
FL = NHL * D        # 512 local features
NCC = C // 128      # 8 contraction chunks over C
NTB = T // 128      # 16 token blocks
NTT = T // 512      # 4 token tiles / qtiles
NEG = -1.0e30


def build():
    nc = bacc.Bacc("TRN2", target_bir_lowering=False, num_devices=8)

    xb0 = nc.dram_tensor("xb0", [512, C], BF16, kind="ExternalInput")
    xtd = nc.dram_tensor("xtd", [NTT, 128, NCC * 512], BF16, kind="ExternalInput")
    wk = nc.dram_tensor("wk", [128, NCC * FL], BF16, kind="ExternalInput")
    wq = nc.dram_tensor("wq", [128, NCC * FL], BF16, kind="ExternalInput")
    wv = nc.dram_tensor("wv", [128, NCC * FL], BF16, kind="ExternalInput")
    wo = nc.dram_tensor("wo", [128, NHP * C], BF16, kind="ExternalInput")
    bqk = nc.dram_tensor("bqk", [128, 2 * NHP], F32, kind="ExternalInput")
    bob = nc.dram_tensor("bob", [128, C], F32, kind="ExternalInput")  # (bo/2+bv@Wo) bcast
    consts = nc.dram_tensor("consts", [128, 256], BF16, kind="ExternalInput")
    zh = nc.dram_tensor("zh", [T // 2, C], F32, kind="ExternalOutput")

    with tile.TileContext(nc) as tc:
        with (
            tc.tile_pool(name="res", bufs=1) as res,
            tc.tile_pool(name="dram", bufs=1, space="DRAM") as dram,
        ):
            # resident: Q^T, K^T bf16 [128, 4hp x 2048tok]; V+ones bf16
            # [128, 16tb x 520]; 0/1 triangle, identity + out-proj consts.
            qt_sb = res.tile([128, NHP * T], BF16)
            kt_sb = res.tile([128, NHP * T], BF16)
            v_sb = res.tile([128, NTB * (NHL * 65)], BF16)
            con_sb = res.tile([128, 256], BF16, name="con")
            wo_sb = res.tile([128, NHP * C], BF16, name="wo_sb")
            bob_sb = res.tile([128, C], F32, name="bob_sb")
            zpart = dram.tile([T, C], F32)
            zreds = [
                dram.tile([128, C], F32, name=f"zred{i}") for i in range(8)
            ]
            idb_sb = con_sb[:, 0:128]
            tri_sb = con_sb[:, 128:256]

            with (
                tc.tile_pool(name="p1c", bufs=1) as p1c,
                tc.tile_pool(name="p1", bufs=4) as p1,
                tc.tile_pool(name="xtp", bufs=2) as xtp,
                tc.tile_pool(name="ytp", bufs=4) as ytp,
                tc.tile_pool(name="ysb_pool", bufs=1) as ysb_pool,
                tc.tile_pool(name="p2", bufs=10) as p2,
                tc.tile_pool(name="norm", bufs=12) as norm,
                tc.tile_pool(name="p3", bufs=10) as p3,
                tc.tile_pool(name="s_ps", bufs=2, space="PSUM") as s_ps_pool,
                tc.tile_pool(name="yu_ps", bufs=2, space="PSUM") as yu_ps_pool,
                tc.tile_pool(name="scr_ps", bufs=2, space="PSUM") as scr_ps_pool,
            ):
                ysb = ysb_pool.tile([128, NHP * T], BF16)

                # ---- preamble DMAs (order = queue order = first-use order) ----
                wk_sb = p1c.tile([128, NCC * FL], BF16, tag="wk")
                wq_sb = p1c.tile([128, NCC * FL], BF16, tag="wq")
                wv_sb = p1c.tile([128, NCC * FL], BF16, tag="wv")
                bqk_sb = p1c.tile([128, 2 * NHP], F32, tag="bqk")
                bk_sb = bqk_sb[:, 0:NHP]
                bq_sb = bqk_sb[:, NHP:2 * NHP]

                xnats = {}
                for tb in range(2):
                    xnats[tb] = p1.tile([128, C], BF16, tag="xnat",
                                        name=f"xnat{tb}")
                nc.sync.dma_start(xnats[0][:], xb0[0:128, :])
                nc.sync.dma_start(con_sb[:], consts[:, :])
                nc.sync.dma_start(bqk_sb[:], bqk[:, :])
                nc.sync.dma_start(xnats[1][:], xb0[128:256, :])

                def x3(x):
                    return x.rearrange("p (c t) -> p c t", c=NCC)

                xts = {}

                def load_xt(tt):
                    xts[tt] = xtp.tile([128, NCC * 512], BF16, tag="xt",
                                       name=f"xt{tt}")
                    if tt == 0:
                        # tokens 0-255 come from on-device transposes (cheap
                        # PE work that burns the clock-ramp window); the
                        # rest streams from the host-side x^T image
                        nc.sync.dma_start(x3(xts[0])[:, :, 256:512],
                                          x3(xtd[0, :, :])[:, :, 256:512])
                    else:
                        nc.sync.dma_start(xts[tt][:], xtd[tt, :, :])

                # weights are host-arranged fb-major, so the first-needed
                # feature-block slices are contiguous full-rate DMAs
                nc.sync.dma_start(wk_sb[:, 0:1024], wk[:, 0:1024])
                nc.sync.dma_start(wq_sb[:, 0:1024], wq[:, 0:1024])
                nc.sync.dma_start(wv_sb[:, 0:2048], wv[:, 0:2048])
                load_xt(0)
                # warm the exp table (hides ~2.7us ACT_TABLE_LOAD)
                warm = p1c.tile([1, 1], F32, tag="warm")
                nc.scalar.activation(warm[:], bqk_sb[0:1, 0:1], AF.Exp)
                nc.sync.dma_start(wk_sb[:, 1024:4096], wk[:, 1024:4096])
                nc.sync.dma_start(wq_sb[:, 1024:4096], wq[:, 1024:4096])
                nc.sync.dma_start(wv_sb[:, 2048:4096], wv[:, 2048:4096])
                load_xt(1)
                nc.sync.dma_start(wo_sb[:], wo[:, :])
                nc.sync.dma_start(bob_sb[:], bob[:, :])

                # ---- pieces: x^T transposes (tile 0), QKV halves ----
                npieces = {tt: 0 for tt in range(NTT)}
                NP_TT = 24  # qkv pieces per tile (16 qk halves + 8 v halves)

                def piece_done(tt):
                    # last piece of tile tt frees its xt buffer: queue the
                    # DMA for tt+2 right here (xtp bufs=2)
                    npieces[tt] += 1
                    full = NP_TT + (2 if tt == 0 else 0)
                    if npieces[tt] == full and tt + 2 < NTT:
                        load_xt(tt + 2)

                def tp_piece(k):
                    # transpose token block k into x^T tile 0
                    xnat = xnats.pop(k)
                    xt = xts[0]
                    for cg in range(NCC // 4):
                        tp_ps = scr_ps_pool.tile([128, 512], BF16, tag="scr",
                                                 name=f"tp{k}_{cg}")
                        for kk in range(4):
                            cc = cg * 4 + kk
                            nc.tensor.transpose(
                                tp_ps[:, kk * 128:(kk + 1) * 128],
                                xnat[:, cc * 128:(cc + 1) * 128], idb_sb[:]
                            )
                        dst = xt[:].rearrange("p (c t) -> p c t", c=NCC)[
                            :, cg * 4:(cg + 1) * 4, k * 128:(k + 1) * 128
                        ]
                        src = tp_ps[:].rearrange("p (k t) -> p k t", k=4)
                        nc.vector.tensor_scalar_add(dst, src, 0.0)
                    piece_done(0)

                def qk_half(tt, fb, th, w_sb, b_sb, dst):
                    # K or Q projection for feature block fb, token half th
                    xt = xts[tt]
                    ps = scr_ps_pool.tile([128, 256], F32, tag="scr",
                                          name=f"ps{tt}_{fb}_{th}")
                    for cc in range(NCC):
                        nc.tensor.matmul(
                            ps[:],
                            w_sb[:, (fb * NCC + cc) * 128:
                                 (fb * NCC + cc + 1) * 128],
                            xt[:, cc * 512 + th * 256: cc * 512 + (th + 1) * 256],
                            start=(cc == 0),
                            stop=(cc == NCC - 1),
                        )
                    nc.vector.tensor_scalar_add(
                        dst[:, fb * T + tt * 512 + th * 256:
                            fb * T + tt * 512 + (th + 1) * 256],
                        ps[:],
                        b_sb[:, fb:fb + 1],
                    )
                    piece_done(tt)

                def v_half(tt, k, fh):
                    # V projection for token block 4tt+k, feature half fh
                    tb = 4 * tt + k
                    xt = xts[tt]
                    ps = scr_ps_pool.tile([128, 256], F32, tag="scr",
                                          name=f"psv{tb}_{fh}")
                    for cc in range(NCC):
                        nc.tensor.matmul(
                            ps[:],
                            xt[:, cc * 512 + k * 128: cc * 512 + (k + 1) * 128],
                            wv_sb[:, (fh * NCC + cc) * 256:
                                  (fh * NCC + cc + 1) * 256],
                            start=(cc == 0),
                            stop=(cc == NCC - 1),
                        )
                    vslice = v_sb[:, tb * (NHL * 65):(tb + 1) * (NHL * 65)]
                    v3 = vslice.rearrange("p (h c) -> p h c", h=NHL)
                    nc.vector.tensor_scalar_add(
                        v3[:, fh * 4:(fh + 1) * 4, 0:D],
                        ps[:].rearrange("p (h d) -> p h d", h=4),
                        0.0,
                    )
                    nc.gpsimd.memset(v3[:, fh * 4:(fh + 1) * 4, D:D + 1], 1.0)
                    piece_done(tt)

                # ---- out-proj + y-transpose + ReduceScatter pieces ----
                # zpart rows chunk-major: chunk c holds tb c then tb 8+c, so
                # each pairwise ReduceScatter chunk is one contiguous block.
                ZROW = {}
                for c in range(8):
                    ZROW[c] = c * 256
                    ZROW[8 + c] = c * 256 + 128

                y_ts = {}

                def fin_piece(qt, tl):
                    # transpose y_t -> ysb feature-major for the out-proj
                    y_t = y_ts[qt]
                    tb = 4 * qt + tl
                    tp = scr_ps_pool.tile([128, 512], BF16, tag="scr",
                                          name=f"ytp{qt}_{tl}")
                    for fc in range(NHP):
                        nc.tensor.transpose(
                            tp[:, fc * 128:(fc + 1) * 128],
                            y_t[:, tl * 512 + fc * 128:
                                tl * 512 + (fc + 1) * 128],
                            idb_sb[:],
                        )
                    nc.vector.tensor_scalar_add(
                        ysb[:].rearrange("p (h t) -> p h t", h=NHP)[
                            :, :, tb * 128:(tb + 1) * 128],
                        tp[:].rearrange("p (h t) -> p h t", h=NHP),
                        0.0,
                    )

                def op_chunk(tb, ct):
                    zrow = ZROW[tb]
                    zps = scr_ps_pool.tile([128, 512], F32, tag="scr",
                                           name=f"z{tb}_{ct}")
                    for cc in range(NHP):
                        nc.tensor.matmul(
                            zps[:],
                            ysb[:, cc * T + tb * 128: cc * T + (tb + 1) * 128],
                            wo_sb[:, cc * C + ct * 512: cc * C + (ct + 1) * 512],
                            start=(cc == 0),
                            stop=(cc == NHP - 1),
                        )
                    z_sb = p3.tile(
                        [128, 512], F32, tag="zsb", name=f"zsb{tb}_{ct}"
                    )
                    nc.vector.tensor_add(
                        z_sb[:], zps[:], bob_sb[:, ct * 512:(ct + 1) * 512]
                    )
                    nc.sync.dma_start(
                        zpart[zrow:zrow + 128, ct * 512:(ct + 1) * 512],
                        z_sb[:],
                    )

                def rs_chunk(c):
                    # (the compiler rejects collectives writing IO tensors,
                    # so stage through zreds and DMA into zh)
                    nc.gpsimd.collective_compute(
                        "ReduceScatter",
                        mybir.AluOpType.add,
                        replica_groups=[[0, 1], [2, 3], [4, 5], [6, 7]],
                        ins=[zpart[c * 256:(c + 1) * 256, :].opt()],
                        outs=[zreds[c].opt()],
                    )
                    nc.sync.dma_start(
                        zh[c * 128:(c + 1) * 128, :], zreds[c][:]
                    )

                # ---- piece queue: dripped into attention units ----
                queue = []
                emitted = [0]

                def drip():
                    if queue:
                        emitted[0] += 1
                        queue.pop(0)()

                def item(fn, *args):
                    def go():
                        fn(*args)
                    return go

                def op_item(tb, ct, c=None):
                    def go():
                        op_chunk(tb, ct)
                        if c is not None:
                            rs_chunk(c)
                    return go

                # queue pieces in dependency order, recording for each
                # attention unit (qt,hp) how many pieces must be emitted
                # before it (its K/Q columns, its V feature half, and for
                # tt0 the x^T transposes).
                prereq = {}
                total = [0]

                def app(ps):
                    queue.extend(ps)
                    total[0] += len(ps)

                def app_qkv(tt):
                    def qk2(fb, w_sb, b_sb, dst):
                        return [item(qk_half, tt, fb, th, w_sb, b_sb, dst)
                                for th in range(2)]

                    if tt == 0:
                        # startup: token-half-major order matching the DMA
                        # stream (tp transposes give tokens 0-255; the
                        # token 256-511 x^T slice lands after wk/wq/wv)
                        app([item(tp_piece, k) for k in range(2)])
                        app([item(qk_half, 0, 0, 0, wk_sb, bk_sb, kt_sb),
                             item(qk_half, 0, 0, 0, wq_sb, bq_sb, qt_sb),
                             item(v_half, 0, 0, 0),
                             item(v_half, 0, 1, 0),
                             item(qk_half, 0, 0, 1, wk_sb, bk_sb, kt_sb),
                             item(qk_half, 0, 0, 1, wq_sb, bq_sb, qt_sb)])
                    else:
                        app(qk2(0, wk_sb, bk_sb, kt_sb)
                            + qk2(0, wq_sb, bq_sb, qt_sb))
                        app([item(v_half, tt, k, 0) for k in range(2)])
                    prereq[(tt, 0)] = total[0]
                    # V k2/k3 are only read by the unit's kg>=1 attnV; the
                    # kg0 drips deliver them, shortening the serial prologue
                    app([item(v_half, tt, k, 0) for k in range(2, 4)])
                    app(qk2(1, wk_sb, bk_sb, kt_sb) + qk2(1, wq_sb, bq_sb, qt_sb))
                    prereq[(tt, 1)] = total[0]
                    app(qk2(2, wk_sb, bk_sb, kt_sb) + qk2(2, wq_sb, bq_sb, qt_sb))
                    app([item(v_half, tt, k, 1) for k in range(4)])
                    prereq[(tt, 2)] = total[0]
                    app(qk2(3, wk_sb, bk_sb, kt_sb) + qk2(3, wq_sb, bq_sb, qt_sb))
                    prereq[(tt, 3)] = total[0]

                for tt in range(NTT):
                    app_qkv(tt)

                # ---- flat cross-unit-pipelined attention ----
                seq = [(qt, hp) for qt in range(NTT) for hp in range(NHP)]
                ustate = {}

                def unit_alloc(ui):
                    qt, hp = seq[ui]
                    if qt not in y_ts:
                        y_ts[qt] = ytp.tile([128, 4 * 512], BF16, tag="yt",
                                            name=f"yt{qt}")
                    ustate[ui] = {
                        "yus": [
                            yu_ps_pool.tile([128, 4 * 65], F32, tag="yu",
                                            name=f"yu{ui}_{i}")
                            for i in range(2)
                        ],
                        "qsl": qt_sb[:, hp * T + qt * 512:
                                     hp * T + (qt + 1) * 512],
                        "sss": {},
                    }

                def emit_scores(ui, kg, hi=None):
                    qt, hp = seq[ui]
                    st = ustate[ui]
                    if hi is None:
                        st["sss"][kg] = [
                            s_ps_pool.tile([128, 1024], F32, tag="s",
                                           name=f"s{ui}_{kg}_{i}")
                            for i in range(2)
                        ]
                        for h2 in range(2):
                            emit_scores(ui, kg, h2)
                        return
                    ss = st["sss"][kg]
                    for c2 in range(2):
                        kb = kg * 2 + c2
                        c = kb - 4 * qt
                        # c==1 writes the full block so the untrimmed exp
                        # never reads unwritten PSUM (extra cols unused)
                        j0 = c * 128 if c >= 2 else 0
                        nc.tensor.matmul(
                            ss[hi][:, c2 * 512 + j0:(c2 + 1) * 512],
                            kt_sb[
                                hi * 64:(hi + 1) * 64,
                                hp * T + kb * 128: hp * T + (kb + 1) * 128,
                            ],
                            st["qsl"][hi * 64:(hi + 1) * 64, j0:],
                            tile_position=(hi * 64, 0),
                            start=True,
                            stop=True,
                        )


                def emit_exp(ui, kg, hi, at):
                    qt, hp = seq[ui]
                    ss = ustate[ui]["sss"][kg]
                    j0r = []
                    for c2 in range(2):
                        c = kg * 2 + c2 - 4 * qt
                        j0r.append(c * 128 if c > 0 else 0)
                    if j0r[0] >= 256:
                        # heavily masked pair: exp valid suffixes only
                        nc.scalar.activation(
                            at[:, j0r[0]:512], ss[hi][:, j0r[0]:512],
                            AF.Exp, scale=0.125,
                        )
                        nc.scalar.activation(
                            at[:, 512 + j0r[1]:1024],
                            ss[hi][:, 512 + j0r[1]:1024],
                            AF.Exp, scale=0.125,
                        )
                    else:
                        nc.scalar.activation(
                            at[:], ss[hi][:], AF.Exp, scale=0.125
                        )
                    # zero the upper triangle of diagonal bands (gpsimd,
                    # SBUF-only) instead of adding -inf before the exp:
                    # keeps the scores->exp chain short and the DVE free.
                    for c2 in range(2):
                        c = kg * 2 + c2 - 4 * qt
                        if 0 <= c <= 3:
                            b0 = c2 * 512 + c * 128
                            nc.gpsimd.tensor_mul(
                                at[:, b0:b0 + 128],
                                at[:, b0:b0 + 128],
                                tri_sb[:],
                            )

                def emit_attnv(ui, kg, hi, at):
                    # Emits the unmasked attnV blocks; returns a closure for
                    # the masked diagonal bands, deferred until after both
                    # his' main work so the serial gpsimd mask multiplies
                    # overlap fat PE work instead of gating it.
                    qt, hp = seq[ui]
                    st = ustate[ui]
                    n_kb = 4 * (qt + 1)
                    h = 2 * hp + hi

                    def mm(c2, kb, qoff):
                        vsl = v_sb[
                            :,
                            kb * (NHL * 65) + h * 65:
                            kb * (NHL * 65) + h * 65 + 65,
                        ]
                        nc.tensor.matmul(
                            st["yus"][hi][:, qoff * 65:(qoff + 1) * 65],
                            at[:, c2 * 512 + qoff * 128:
                               c2 * 512 + (qoff + 1) * 128],
                            vsl,
                            start=(kb == 0 and qoff == 3),
                            stop=(kb == n_kb - 1),
                        )

                    bands = []
                    for c2 in range(2):
                        kb = kg * 2 + c2
                        # qoff descending: the very first emitted write of
                        # the unit carries start=True (clears the yu bank)
                        qlo = max(kb - 4 * qt, 0)
                        for qoff in range(3, qlo - 1, -1):
                            if qoff == kb - 4 * qt:
                                bands.append((c2, kb, qoff))
                            else:
                                mm(c2, kb, qoff)

                    def emit_bands():
                        for c2, kb, qoff in bands:
                            mm(c2, kb, qoff)
                    return emit_bands

                def normalize_hi(ui, hi):
                    # emitted right after this hi's last attnV so the yu
                    # buffer frees (and y_t fills) as early as possible.
                    # The very last unit's multiplies run on the (by then
                    # idle) Act engine so the drain phase isn't serialized
                    # on the DVE behind the y-transpose copies.
                    qt, hp = seq[ui]
                    st = ustate[ui]
                    y_t = y_ts[qt]
                    yu3 = st["yus"][hi][:].rearrange("p (b c) -> p b c", b=4)
                    rcp = norm.tile([128, 4], F32, tag="rcp")
                    nc.vector.reciprocal(
                        rcp[:].unsqueeze(2), yu3[:, :, 64:65]
                    )
                    h = 2 * hp + hi
                    for qoff in range(4):
                        dst = y_t[:, qoff * 512 + h * 64:
                                  qoff * 512 + h * 64 + 64]
                        src = st["yus"][hi][:, qoff * 65: qoff * 65 + 64]
                        if ui == len(seq) - 1 and hi == 1:
                            nc.scalar.activation(
                                dst, src, AF.Copy,
                                scale=rcp[:, qoff:qoff + 1],
                            )
                        else:
                            nc.vector.tensor_scalar_mul(
                                dst, src, rcp[:, qoff:qoff + 1],
                            )

                def drain_to(n):
                    while emitted[0] < n and queue:
                        drip()

                for ui, (qt, hp) in enumerate(seq):
                    n_kg = 2 * (qt + 1)
                    drain_to(prereq[(qt, hp)])
                    if ui == 0:
                        unit_alloc(0)
                        emit_scores(0, 0)
                    for kg in range(n_kg):
                        last_kg = kg == n_kg - 1
                        if last_kg and ui + 1 < len(seq):
                            drain_to(prereq[seq[ui + 1]])
                        # qt>=2 windows must not drain the queue early:
                        # their per-kg piece deficit is ~1 piece, so drip
                        # once per kg there and twice per kg before.
                        drips = (2 if qt < 2 else 1)
                        bandfns = []
                        for hi in range(2):
                            at = p2.tile([128, 1024], BF16, tag="attn")
                            emit_exp(ui, kg, hi, at)
                            if not last_kg:
                                if hi == 0:
                                    ustate[ui]["sss"][kg + 1] = [
                                        s_ps_pool.tile(
                                            [128, 1024], F32, tag="s",
                                            name=f"s{ui}_{kg + 1}_{i}")
                                        for i in range(2)
                                    ]
                                emit_scores(ui, kg + 1, hi)
                            elif ui + 1 < len(seq):
                                if hi == 0:
                                    unit_alloc(ui + 1)
                                    ustate[ui + 1]["sss"][0] = [
                                        s_ps_pool.tile(
                                            [128, 1024], F32, tag="s",
                                            name=f"s{ui + 1}_0_{i}")
                                        for i in range(2)
                                    ]
                                emit_scores(ui + 1, 0, hi)
                            # drip BEFORE the attnV batch: the ~12 attnV
                            # matmuls all wait on the exp semaphore, and the
                            # PE wait-queue is only 4 deep — emitting them
                            # first would block the sequencer before the
                            # ready fill work behind them could dispatch
                            if hi < drips:
                                drip()
                            bandfns.append(emit_attnv(ui, kg, hi, at))
                        for fn in bandfns:
                            fn()
                        if last_kg:
                            normalize_hi(ui, 0)
                            normalize_hi(ui, 1)
                    if hp == NHP - 1:
                        # unit-set (qtile) complete: queue its y-transposes
                        # and, once qt>=1 transposes exist, the out-proj
                        # chunks that only need earlier qtiles.
                        app([item(fin_piece, qt, tl) for tl in range(4)])
                        if qt == 2:
                            app([op_item(tb, ct)
                                 for tb in range(8) for ct in range(2)])
                            app([op_item(8 + c, ct, c if ct else None)
                                 for c in range(2) for ct in range(2)])
                        if qt == 3:
                            # held-back chunks fill the normalize/fin(3)
                            # latency at the drain boundary
                            app([op_item(8 + c, ct, c if ct else None)
                                 for c in range(2, 4) for ct in range(2)])
                while queue:
                    drip()
                for i in range(4):
                    op_chunk(12 + i, 0)
                    if i < 3:
                        op_chunk(12 + i, 1)
                        rs_chunk(4 + i)
                # final chunk as two half-width PSUM groups: the first
                # half's bias+DMA overlap the second half's matmuls and
                # the tail DMA shrinks to 256 columns
                zrow = ZROW[15]
                for ph in range(2):
                    zps = scr_ps_pool.tile([128, 256], F32, tag="scr",
                                           name=f"z15_1{ph}")
                    c0 = 512 + ph * 256
                    for cc in range(NHP):
                        nc.tensor.matmul(
                            zps[:],
                            ysb[:, cc * T + 15 * 128: cc * T + 16 * 128],
                            wo_sb[:, cc * C + c0: cc * C + c0 + 256],
                            start=(cc == 0),
                            stop=(cc == NHP - 1),
                        )
                    z_sb = p3.tile([128, 256], F32, tag="zsb",
                                   name=f"zsb15_1{ph}")
                    nc.vector.tensor_add(
                        z_sb[:], zps[:], bob_sb[:, c0:c0 + 256]
                    )
                    nc.sync.dma_start(
                        zpart[zrow:zrow + 128, c0:c0 + 256], z_sb[:]
                    )
                rs_chunk(7)

    nc.compile()
    return nc


_NC_CACHE = None


def _get_nc():
    global _NC_CACHE
    if _NC_CACHE is None:
        _NC_CACHE = build()
    return _NC_CACHE


def _in_maps(x, Wqkv, bqkv, Wo, bo):
    x = np.ascontiguousarray(np.asarray(x, dtype=np.float32))
    Wqkv = np.ascontiguousarray(np.asarray(Wqkv, dtype=np.float32))
    bqkv = np.asarray(bqkv, dtype=np.float32)
    Wo = np.ascontiguousarray(np.asarray(Wo, dtype=np.float32))
    bo = np.asarray(bo, dtype=np.float32)

    from ml_dtypes import bfloat16
    identb = np.eye(128, dtype=bfloat16)
    i_ = np.arange(128, dtype=np.int64)[:, None]
    j_ = np.arange(128, dtype=np.int64)[None, :]
    tri01 = np.where(i_ > j_, np.float32(0.0), np.float32(1.0)).astype(bfloat16)
    consts = np.concatenate([identb, tri01], axis=1)

    # x^T SBUF images: [tt, 128, cc*512] with xtd[tt][p][cc*512+t] =
    # x[tt*512+t, cc*128+p]
    xts = {}
    xb0s = {}
    for b in range(B):
        xt = x[b].T.astype(bfloat16)              # [C, T]
        xt = xt.reshape(NCC, 128, NTT, 512)
        xts[b] = np.ascontiguousarray(
            xt.transpose(2, 1, 0, 3).reshape(NTT, 128, NCC * 512))
        xb0s[b] = np.ascontiguousarray(x[b][:512]).astype(bfloat16)

    def w_img(W, fblk):
        # [C, F] -> [128, (F//fblk)*NCC*fblk] fb-major partition-major image:
        # col ((fb*NCC + cc)*fblk + j) <- W[cc*128 + p, fb*fblk + j]
        F = W.shape[1]
        nfb = F // fblk
        w = W.reshape(NCC, 128, nfb, fblk)          # [cc, p, fb, j]
        return np.ascontiguousarray(
            w.transpose(1, 2, 0, 3).reshape(128, F * NCC)
        ).astype(bfloat16)

    in_maps = []
    for core in range(8):
        b, hh = core // 2, core % 2
        sl = slice(hh * FL, (hh + 1) * FL)
        bv_loc = bqkv[2 * C:][sl]
        wo_loc = np.ascontiguousarray(Wo[sl, :])
        # V bias folded into output bias: attn rows sum to 1 after normalize
        bo_loc = bo * 0.5 + bv_loc @ wo_loc
        # wo image over its 4 cc chunks of 128 (FL=512 rows)
        wo_img = np.ascontiguousarray(
            wo_loc.reshape(NHP, 128, C).transpose(1, 0, 2).reshape(128, NHP * C)
        ).astype(bfloat16)
        in_maps.append({
            "xb0": xb0s[b],
            "xtd": xts[b],
            "wq": w_img(np.ascontiguousarray(Wqkv[:, 0 * C:1 * C][:, sl]), 128),
            "wk": w_img(np.ascontiguousarray(Wqkv[:, 1 * C:2 * C][:, sl]), 128),
            "wv": w_img(np.ascontiguousarray(Wqkv[:, 2 * C:3 * C][:, sl]), 256),
            "wo": wo_img,
            "bqk": np.ascontiguousarray(np.concatenate([
                bqkv[1 * C:2 * C][sl].reshape(NHP, 128).T,
                bqkv[0 * C:1 * C][sl].reshape(NHP, 128).T,
            ], axis=1)),
            "bob": np.broadcast_to(bo_loc[None, :], (128, C)).copy(),
            "consts": consts,
        })

    return in_maps


def _assemble(res):
    out = np.empty((B, T, C), dtype=np.float32)
    for b in range(B):
        out[b, : T // 2] = res.results[2 * b]["zh"]
        out[b, T // 2:] = res.results[2 * b + 1]["zh"]
    return out


def kernel(x, Wqkv, bqkv, Wo, bo):
    in_maps = _in_maps(x, Wqkv, bqkv, Wo, bo)
    res = run_bass_kernel_spmd(_get_nc(), in_maps, core_ids=list(range(8)))
    return _assemble(res)


def run_traced(x, Wqkv, bqkv, Wo, bo, trace_cores=None):
    in_maps = _in_maps(x, Wqkv, bqkv, Wo, bo)
    res = run_bass_kernel_spmd(
        _get_nc(), in_maps, core_ids=list(range(8)), trace=True,
        trace_cores=trace_cores,
    )
    return res


# revision 95
# speedup vs baseline: 1.0015x; 1.0015x over previous
"""Causal multi-head attention block (QKV proj -> causal MHA -> out proj) on 8 Trainium2
cores.

Sharding: core = b*2 + hh handles batch b (of 4) and head-half hh (8 of 16 heads),
computing attention for its heads over the full sequence, then a partial output
projection over its 512 y-channels for all 2048 tokens. A pairwise ReduceScatter
([0,1],[2,3],...) sums the two partials of each batch and leaves each core with its
token-half of the final output.

Host-side prep: x^T is laid out as the exact SBUF image (the device transposes
only tokens 0-255, providing cheap PE work that burns the clock-ramp window
while the big DMAs stream in); weights are pre-rearranged to fb-major
partition-major images so the first-needed slices are contiguous full-rate
DMAs; the V bias is folded into the output bias.

Device structure per core:
  - All work is cut into "pieces" (QKV projection halves, x^T transposes,
    y-transposes, out-projection chunks) that are dripped into the attention
    inner loop, which is otherwise Act-bound (softmax exp chain).
  - The attention pipeline is flat across (qtile, head-pair) units: the next
    unit's first scores are emitted inside the previous unit's last key-group
    iteration, so the Act engine never drains at unit boundaries.
  - Causal masking multiplies the diagonal 128x128 bands of the attention
    weights by a 0/1 triangle on the otherwise-idle GPSIMD engine (SBUF only),
    keeping the scores->exp chain free of extra hops.
  - attnV computes y transposed (queries on PSUM partitions, via a ones column
    in V for the softmax denominator) so normalization is a per-partition
    tensor_scalar multiply; y is transposed back on the PE for the out-proj.

dtypes: bf16 matmul operands everywhere; PSUM accumulation f32; the softmax
denominator is accumulated from the same bf16 weights, so normalization is
exact w.r.t. rounding.
"""

import numpy as np

import concourse.bass as bass
import concourse.tile as tile
from concourse import bacc, mybir
from concourse.bass_utils import run_bass_kernel_spmd

F32 = mybir.dt.float32
BF16 = mybir.dt.bfloat16
AF = mybir.ActivationFunctionType

B, T, C, H = 4, 2048, 1024, 16
D = C // H          # 64
NHL = H // 2        # 8 local heads per core
NHP = NHL // 2      # 4 local head pairs
FL = NHL * D        # 512 local features
NCC = C // 128      # 8 contraction chunks over C
NTB = T // 128      # 16 token blocks
NTT = T // 512      # 4 token tiles / qtiles
NEG = -1.0e30


def build():
    nc = bacc.Bacc("TRN2", target_bir_lowering=False, num_devices=8)

    xb0 = nc.dram_tensor("xb0", [512, C], BF16, kind="ExternalInput")
    xtd = nc.dram_tensor("xtd", [NTT, 128, NCC * 512], BF16, kind="ExternalInput")
    wk = nc.dram_tensor("wk", [128, NCC * FL], BF16, kind="ExternalInput")
    wq = nc.dram_tensor("wq", [128, NCC * FL], BF16, kind="ExternalInput")
    wv = nc.dram_tensor("wv", [128, NCC * FL], BF16, kind="ExternalInput")
    wo = nc.dram_tensor("wo", [128, NHP * C], BF16, kind="ExternalInput")
    bqk = nc.dram_tensor("bqk", [128, 2 * NHP], F32, kind="ExternalInput")
    bob = nc.dram_tensor("bob", [128, C], F32, kind="ExternalInput")  # (bo/2+bv@Wo) bcast
    consts = nc.dram_tensor("consts", [128, 256], BF16, kind="ExternalInput")
    zh = nc.dram_tensor("zh", [T // 2, C], F32, kind="ExternalOutput")

    with tile.TileContext(nc) as tc:
        with (
            tc.tile_pool(name="res", bufs=1) as res,
            tc.tile_pool(name="dram", bufs=1, space="DRAM") as dram,
        ):
            # resident: Q^T, K^T bf16 [128, 4hp x 2048tok]; V+ones bf16
            # [128, 16tb x 520]; 0/1 triangle, identity + out-proj consts.
            qt_sb = res.tile([128, NHP * T], BF16)
            kt_sb = res.tile([128, NHP * T], BF16)
            v_sb = res.tile([128, NTB * (NHL * 65)], BF16)
            con_sb = res.tile([128, 256], BF16, name="con")
            wo_sb = res.tile([128, NHP * C], BF16, name="wo_sb")
            bob_sb = res.tile([128, C], F32, name="bob_sb")
            zpart = dram.tile([T, C], F32)
            zreds = [
                dram.tile([128, C], F32, name=f"zred{i}") for i in range(8)
            ]
            idb_sb = con_sb[:, 0:128]
            tri_sb = con_sb[:, 128:256]

            with (
                tc.tile_pool(name="p1c", bufs=1) as p1c,
                tc.tile_pool(name="p1", bufs=4) as p1,
                tc.tile_pool(name="xtp", bufs=2) as xtp,
                tc.tile_pool(name="ytp", bufs=4) as ytp,
                tc.tile_pool(name="ysb_pool", bufs=1) as ysb_pool,
                tc.tile_pool(name="p2", bufs=10) as p2,
                tc.tile_pool(name="norm", bufs=12) as norm,
                tc.tile_pool(name="p3", bufs=10) as p3,
                tc.tile_pool(name="s_ps", bufs=2, space="PSUM") as s_ps_pool,
                tc.tile_pool(name="yu_ps", bufs=2, space="PSUM") as yu_ps_pool,
                tc.tile_pool(name="scr_ps", bufs=2, space="PSUM") as scr_ps_pool,
            ):
                ysb = ysb_pool.tile([128, NHP * T], BF16)

                # ---- preamble DMAs (order = queue order = first-use order) ----
                wk_sb = p1c.tile([128, NCC * FL], BF16, tag="wk")
                wq_sb = p1c.tile([128, NCC * FL], BF16, tag="wq")
                wv_sb = p1c.tile([128, NCC * FL], BF16, tag="wv")
                bqk_sb = p1c.tile([128, 2 * NHP], F32, tag="bqk")
                bk_sb = bqk_sb[:, 0:NHP]
                bq_sb = bqk_sb[:, NHP:2 * NHP]

                xnats = {}
                for tb in range(2):
                    xnats[tb] = p1.tile([128, C], BF16, tag="xnat",
                                        name=f"xnat{tb}")
                nc.sync.dma_start(xnats[0][:], xb0[0:128, :])
                nc.sync.dma_start(con_sb[:], consts[:, :])
                nc.sync.dma_start(bqk_sb[:], bqk[:, :])
                nc.sync.dma_start(xnats[1][:], xb0[128:256, :])

                def x3(x):
                    return x.rearrange("p (c t) -> p c t", c=NCC)

                xts = {}

                def load_xt(tt):
                    xts[tt] = xtp.tile([128, NCC * 512], BF16, tag="xt",
                                       name=f"xt{tt}")
                    if tt == 0:
                        # tokens 0-255 come from on-device transposes (cheap
                        # PE work that burns the clock-ramp window); the
                        # rest streams from the host-side x^T image
                        nc.sync.dma_start(x3(xts[0])[:, :, 256:512],
                                          x3(xtd[0, :, :])[:, :, 256:512])
                    else:
                        nc.sync.dma_start(xts[tt][:], xtd[tt, :, :])

                # weights are host-arranged fb-major, so the first-needed
                # feature-block slices are contiguous full-rate DMAs
                nc.sync.dma_start(wk_sb[:, 0:1024], wk[:, 0:1024])
                nc.sync.dma_start(wq_sb[:, 0:1024], wq[:, 0:1024])
                nc.sync.dma_start(wv_sb[:, 0:2048], wv[:, 0:2048])
                load_xt(0)
                # warm the exp table (hides ~2.7us ACT_TABLE_LOAD)
                warm = p1c.tile([1, 1], F32, tag="warm")
                nc.scalar.activation(warm[:], bqk_sb[0:1, 0:1], AF.Exp)
                nc.sync.dma_start(wk_sb[:, 1024:4096], wk[:, 1024:4096])
                nc.sync.dma_start(wq_sb[:, 1024:4096], wq[:, 1024:4096])
                nc.sync.dma_start(wv_sb[:, 2048:4096], wv[:, 2048:4096])
                load_xt(1)
                nc.sync.dma_start(wo_sb[:], wo[:, :])
                nc.sync.dma_start(bob_sb[:], bob[:, :])

                # ---- pieces: x^T transposes (tile 0), QKV halves ----
                npieces = {tt: 0 for tt in range(NTT)}
                NP_TT = 24  # qkv pieces per tile (16 qk halves + 8 v halves)

                def piece_done(tt):
                    # last piece of tile tt frees its xt buffer: queue the
                    # DMA for tt+2 right here (xtp bufs=2)
                    npieces[tt] += 1
                    full = NP_TT + (2 if tt == 0 else 0)
                    if npieces[tt] == full and tt + 2 < NTT:
                        load_xt(tt + 2)

                def tp_piece(k):
                    # transpose token block k into x^T tile 0
                    xnat = xnats.pop(k)
                    xt = xts[0]
                    for cg in range(NCC // 4):
                        tp_ps = scr_ps_pool.tile([128, 512], BF16, tag="scr",
                                                 name=f"tp{k}_{cg}")
                        for kk in range(4):
                            cc = cg * 4 + kk
                            nc.tensor.transpose(
                                tp_ps[:, kk * 128:(kk + 1) * 128],
                                xnat[:, cc * 128:(cc + 1) * 128], idb_sb[:]
                            )
                        dst = xt[:].rearrange("p (c t) -> p c t", c=NCC)[
                            :, cg * 4:(cg + 1) * 4, k * 128:(k + 1) * 128
                        ]
                        src = tp_ps[:].rearrange("p (k t) -> p k t", k=4)
                        nc.vector.tensor_scalar_add(dst, src, 0.0)
                    piece_done(0)

                def qk_half(tt, fb, th, w_sb, b_sb, dst):
                    # K or Q projection for feature block fb, token half th
                    xt = xts[tt]
                    ps = scr_ps_pool.tile([128, 256], F32, tag="scr",
                                          name=f"ps{tt}_{fb}_{th}")
                    for cc in range(NCC):
                        nc.tensor.matmul(
                            ps[:],
                            w_sb[:, (fb * NCC + cc) * 128:
                                 (fb * NCC + cc + 1) * 128],
                            xt[:, cc * 512 + th * 256: cc * 512 + (th + 1) * 256],
                            start=(cc == 0),
                            stop=(cc == NCC - 1),
                        )
                    nc.vector.tensor_scalar_add(
                        dst[:, fb * T + tt * 512 + th * 256:
                            fb * T + tt * 512 + (th + 1) * 256],
                        ps[:],
                        b_sb[:, fb:fb + 1],
                    )
                    piece_done(tt)

                def v_half(tt, k, fh):
                    # V projection for token block 4tt+k, feature half fh
                    tb = 4 * tt + k
                    xt = xts[tt]
                    ps = scr_ps_pool.tile([128, 256], F32, tag="scr",
                                          name=f"psv{tb}_{fh}")
                    for cc in range(NCC):
                        nc.tensor.matmul(
                            ps[:],
                            xt[:, cc * 512 + k * 128: cc * 512 + (k + 1) * 128],
                            wv_sb[:, (fh * NCC + cc) * 256:
                                  (fh * NCC + cc + 1) * 256],
                            start=(cc == 0),
                            stop=(cc == NCC - 1),
                        )
                    vslice = v_sb[:, tb * (NHL * 65):(tb + 1) * (NHL * 65)]
                    v3 = vslice.rearrange("p (h c) -> p h c", h=NHL)
                    nc.vector.tensor_scalar_add(
                        v3[:, fh * 4:(fh + 1) * 4, 0:D],
                        ps[:].rearrange("p (h d) -> p h d", h=4),
                        0.0,
                    )
                    nc.gpsimd.memset(v3[:, fh * 4:(fh + 1) * 4, D:D + 1], 1.0)
                    piece_done(tt)

                # ---- out-proj + y-transpose + ReduceScatter pieces ----
                # zpart rows chunk-major: chunk c holds tb c then tb 8+c, so
                # each pairwise ReduceScatter chunk is one contiguous block.
                ZROW = {}
                for c in range(8):
                    ZROW[c] = c * 256
                    ZROW[8 + c] = c * 256 + 128

                y_ts = {}

                def fin_piece(qt, tl):
                    # transpose y_t -> ysb feature-major for the out-proj
                    y_t = y_ts[qt]
                    tb = 4 * qt + tl
                    tp = scr_ps_pool.tile([128, 512], BF16, tag="scr",
                                          name=f"ytp{qt}_{tl}")
                    for fc in range(NHP):
                        nc.tensor.transpose(
                            tp[:, fc * 128:(fc + 1) * 128],
                            y_t[:, tl * 512 + fc * 128:
                                tl * 512 + (fc + 1) * 128],
                            idb_sb[:],
                        )
                    nc.vector.tensor_scalar_add(
                        ysb[:].rearrange("p (h t) -> p h t", h=NHP)[
                            :, :, tb * 128:(tb + 1) * 128],
                        tp[:].rearrange("p (h t) -> p h t", h=NHP),
                        0.0,
                    )

                def op_chunk(tb, ct):
                    zrow = ZROW[tb]
                    zps = scr_ps_pool.tile([128, 512], F32, tag="scr",
                                           name=f"z{tb}_{ct}")
                    for cc in range(NHP):
                        nc.tensor.matmul(
                            zps[:],
                            ysb[:, cc * T + tb * 128: cc * T + (tb + 1) * 128],
                            wo_sb[:, cc * C + ct * 512: cc * C + (ct + 1) * 512],
                            start=(cc == 0),
                            stop=(cc == NHP - 1),
                        )
                    z_sb = p3.tile(
                        [128, 512], F32, tag="zsb", name=f"zsb{tb}_{ct}"
                    )
                    nc.vector.tensor_add(
                        z_sb[:], zps[:], bob_sb[:, ct * 512:(ct + 1) * 512]
                    )
                    nc.sync.dma_start(
                        zpart[zrow:zrow + 128, ct * 512:(ct + 1) * 512],
                        z_sb[:],
                    )

                def rs_chunk(c):
                    # (the compiler rejects collectives writing IO tensors,
                    # so stage through zreds and DMA into zh)
                    nc.gpsimd.collective_compute(
                        "ReduceScatter",
                        mybir.AluOpType.add,
                        replica_groups=[[0, 1], [2, 3], [4, 5], [6, 7]],
                        ins=[zpart[c * 256:(c + 1) * 256, :].opt()],
                        outs=[zreds[c].opt()],
                    )
                    nc.sync.dma_start(
                        zh[c * 128:(c + 1) * 128, :], zreds[c][:]
                    )

                # ---- piece queue: dripped into attention units ----
                queue = []
                emitted = [0]

                def drip():
                    if queue:
                        emitted[0] += 1
                        queue.pop(0)()

                def item(fn, *args):
                    def go():
                        fn(*args)
                    return go

                def op_item(tb, ct, c=None):
                    def go():
                        op_chunk(tb, ct)
                        if c is not None:
                            rs_chunk(c)
                    return go

                # queue pieces in dependency order, recording for each
                # attention unit (qt,hp) how many pieces must be emitted
                # before it (its K/Q columns, its V feature half, and for
                # tt0 the x^T transposes).
                prereq = {}
                total = [0]

                def app(ps):
                    queue.extend(ps)
                    total[0] += len(ps)

                def app_qkv(tt):
                    def qk2(fb, w_sb, b_sb, dst):
                        return [item(qk_half, tt, fb, th, w_sb, b_sb, dst)
                                for th in range(2)]

                    if tt == 0:
                        # startup: token-half-major order matching the DMA
                        # stream (tp transposes give tokens 0-255; the
                        # token 256-511 x^T slice lands after wk/wq/wv)
                        app([item(tp_piece, k) for k in range(2)])
                        app([item(qk_half, 0, 0, 0, wk_sb, bk_sb, kt_sb),
                             item(qk_half, 0, 0, 0, wq_sb, bq_sb, qt_sb),
                             item(v_half, 0, 0, 0),
                             item(v_half, 0, 1, 0),
                             item(qk_half, 0, 0, 1, wk_sb, bk_sb, kt_sb),
                             item(qk_half, 0, 0, 1, wq_sb, bq_sb, qt_sb)])
                    else:
                        app(qk2(0, wk_sb, bk_sb, kt_sb)
                            + qk2(0, wq_sb, bq_sb, qt_sb))
                        app([item(v_half, tt, k, 0) for k in range(2)])
                    prereq[(tt, 0)] = total[0]
                    # V k2/k3 are only read by the unit's kg>=1 attnV; the
                    # kg0 drips deliver them, shortening the serial prologue
                    app([item(v_half, tt, k, 0) for k in range(2, 4)])
                    app(qk2(1, wk_sb, bk_sb, kt_sb) + qk2(1, wq_sb, bq_sb, qt_sb))
                    prereq[(tt, 1)] = total[0]
                    app(qk2(2, wk_sb, bk_sb, kt_sb) + qk2(2, wq_sb, bq_sb, qt_sb))
                    app([item(v_half, tt, k, 1) for k in range(4)])
                    prereq[(tt, 2)] = total[0]
                    app(qk2(3, wk_sb, bk_sb, kt_sb) + qk2(3, wq_sb, bq_sb, qt_sb))
                    prereq[(tt, 3)] = total[0]

                for tt in range(NTT):
                    app_qkv(tt)

                # ---- flat cross-unit-pipelined attention ----
                seq = [(qt, hp) for qt in range(NTT) for hp in range(NHP)]
                ustate = {}

                def unit_alloc(ui):
                    qt, hp = seq[ui]
                    if qt not in y_ts:
                        y_ts[qt] = ytp.tile([128, 4 * 512], BF16, tag="yt",
                                            name=f"yt{qt}")
                    ustate[ui] = {
                        "yus": [
                            yu_ps_pool.tile([128, 4 * 65], F32, tag="yu",
                                            name=f"yu{ui}_{i}")
                            for i in range(2)
                        ],
                        "qsl": qt_sb[:, hp * T + qt * 512:
                                     hp * T + (qt + 1) * 512],
                        "sss": {},
                    }

                def emit_scores(ui, kg, hi=None):
                    qt, hp = seq[ui]
                    st = ustate[ui]
                    if hi is None:
                        st["sss"][kg] = [
                            s_ps_pool.tile([128, 1024], F32, tag="s",
                                           name=f"s{ui}_{kg}_{i}")
                            for i in range(2)
                        ]
                        for h2 in range(2):
                            emit_scores(ui, kg, h2)
                        return
                    ss = st["sss"][kg]
                    for c2 in range(2):
                        kb = kg * 2 + c2
                        c = kb - 4 * qt
                        # c==1 writes the full block so the untrimmed exp
                        # never reads unwritten PSUM (extra cols unused)
                        j0 = c * 128 if c >= 2 else 0
                        nc.tensor.matmul(
                            ss[hi][:, c2 * 512 + j0:(c2 + 1) * 512],
                            kt_sb[
                                hi * 64:(hi + 1) * 64,
                                hp * T + kb * 128: hp * T + (kb + 1) * 128,
                            ],
                            st["qsl"][hi * 64:(hi + 1) * 64, j0:],
                            tile_position=(hi * 64, 0),
                            start=True,
                            stop=True,
                        )


                def emit_exp(ui, kg, hi, at):
                    qt, hp = seq[ui]
                    ss = ustate[ui]["sss"][kg]
                    j0r = []
                    for c2 in range(2):
                        c = kg * 2 + c2 - 4 * qt
                        j0r.append(c * 128 if c > 0 else 0)
                    if j0r[0] >= 256:
                        # heavily masked pair: exp valid suffixes only
                        nc.scalar.activation(
                            at[:, j0r[0]:512], ss[hi][:, j0r[0]:512],
                            AF.Exp, scale=0.125,
                        )
                        nc.scalar.activation(
                            at[:, 512 + j0r[1]:1024],
                            ss[hi][:, 512 + j0r[1]:1024],
                            AF.Exp, scale=0.125,
                        )
                    else:
                        nc.scalar.activation(
                            at[:], ss[hi][:], AF.Exp, scale=0.125
                        )
                    # zero the upper triangle of diagonal bands (gpsimd,
                    # SBUF-only) instead of adding -inf before the exp:
                    # keeps the scores->exp chain short and the DVE free.
                    for c2 in range(2):
                        c = kg * 2 + c2 - 4 * qt
                        if 0 <= c <= 3:
                            b0 = c2 * 512 + c * 128
                            nc.gpsimd.tensor_mul(
                                at[:, b0:b0 + 128],
                                at[:, b0:b0 + 128],
                                tri_sb[:],
                            )

                def emit_attnv(ui, kg, hi, at):
                    # Emits the unmasked attnV blocks; returns a closure for
                    # the masked diagonal bands, deferred until after both
                    # his' main work so the serial gpsimd mask multiplies
                    # overlap fat PE work instead of gating it.
                    qt, hp = seq[ui]
                    st = ustate[ui]
                    n_kb = 4 * (qt + 1)
                    h = 2 * hp + hi

                    def mm(c2, kb, qoff):
                        vsl = v_sb[
                            :,
                            kb * (NHL * 65) + h * 65:
                            kb * (NHL * 65) + h * 65 + 65,
                        ]
                        nc.tensor.matmul(
                            st["yus"][hi][:, qoff * 65:(qoff + 1) * 65],
                            at[:, c2 * 512 + qoff * 128:
                               c2 * 512 + (qoff + 1) * 128],
                            vsl,
                            start=(kb == 0 and qoff == 3),
                            stop=(kb == n_kb - 1),
                        )

                    bands = []
                    for c2 in range(2):
                        kb = kg * 2 + c2
                        # qoff descending: the very first emitted write of
                        # the unit carries start=True (clears the yu bank)
                        qlo = max(kb - 4 * qt, 0)
                        for qoff in range(3, qlo - 1, -1):
                            if qoff == kb - 4 * qt:
                                bands.append((c2, kb, qoff))
                            else:
                                mm(c2, kb, qoff)

                    def emit_bands():
                        for c2, kb, qoff in bands:
                            mm(c2, kb, qoff)
                    return emit_bands

                def normalize_hi(ui, hi):
                    # emitted right after this hi's last attnV so the yu
                    # buffer frees (and y_t fills) as early as possible.
                    # The very last unit's multiplies run on the (by then
                    # idle) Act engine so the drain phase isn't serialized
                    # on the DVE behind the y-transpose copies.
                    qt, hp = seq[ui]
                    st = ustate[ui]
                    y_t = y_ts[qt]
                    yu3 = st["yus"][hi][:].rearrange("p (b c) -> p b c", b=4)
                    rcp = norm.tile([128, 4], F32, tag="rcp")
                    nc.vector.reciprocal(
                        rcp[:].unsqueeze(2), yu3[:, :, 64:65]
                    )
                    h = 2 * hp + hi
                    for qoff in range(4):
                        dst = y_t[:, qoff * 512 + h * 64:
                                  qoff * 512 + h * 64 + 64]
                        src = st["yus"][hi][:, qoff * 65: qoff * 65 + 64]
                        if ui == len(seq) - 1 and hi == 1:
                            nc.scalar.activation(
                                dst, src, AF.Copy,
                                scale=rcp[:, qoff:qoff + 1],
                            )
                        else:
                            nc.vector.tensor_scalar_mul(
                                dst, src, rcp[:, qoff:qoff + 1],
                            )

                def drain_to(n):
                    while emitted[0] < n and queue:
                        drip()

                for ui, (qt, hp) in enumerate(seq):
                    n_kg = 2 * (qt + 1)
                    drain_to(prereq[(qt, hp)])
                    if ui == 0:
                        unit_alloc(0)
                        emit_scores(0, 0)
                    for kg in range(n_kg):
                        last_kg = kg == n_kg - 1
                        if last_kg and ui + 1 < len(seq):
                            drain_to(prereq[seq[ui + 1]])
                        # qt>=2 windows must not drain the queue early:
                        # their per-kg piece deficit is ~1 piece, so drip
                        # once per kg there and twice per kg before.
                        drips = (2 if qt < 2 else 1)
                        bandfns = []
                        for hi in range(2):
                            at = p2.tile([128, 1024], BF16, tag="attn")
                            emit_exp(ui, kg, hi, at)
                            if not last_kg:
                                if hi == 0:
                                    ustate[ui]["sss"][kg + 1] = [
                                        s_ps_pool.tile(
                                            [128, 1024], F32, tag="s",
                                            name=f"s{ui}_{kg + 1}_{i}")
                                        for i in range(2)
                                    ]
                                emit_scores(ui, kg + 1, hi)
                            elif ui + 1 < len(seq):
                                if hi == 0:
                                    unit_alloc(ui + 1)
                                    ustate[ui + 1]["sss"][0] = [
                                        s_ps_pool.tile(
                                            [128, 1024], F32, tag="s",
                                            name=f"s{ui + 1}_0_{i}")
                                        for i in range(2)
                                    ]
                                emit_scores(ui + 1, 0, hi)
                            # drip BEFORE the attnV batch: the ~12 attnV
                            # matmuls all wait on the exp semaphore, and the
                            # PE wait-queue is only 4 deep — emitting them
                            # first would block the sequencer before the
                            # ready fill work behind them could dispatch
                            if hi < drips:
                                drip()
                            bandfns.append(emit_attnv(ui, kg, hi, at))
                        for fn in bandfns:
                            fn()
                        if last_kg:
                            normalize_hi(ui, 0)
                            normalize_hi(ui, 1)
                    if hp == NHP - 1:
                        # unit-set (qtile) complete: queue its y-transposes
                        # and, once qt>=1 transposes exist, the out-proj
                        # chunks that only need earlier qtiles.
                        app([item(fin_piece, qt, tl) for tl in range(4)])
                        if qt == 2:
                            app([op_item(tb, ct)
                                 for tb in range(8) for ct in range(2)])
                            app([op_item(8 + c, ct, c if ct else None)
                                 for c in range(2) for ct in range(2)])
                        if qt == 3:
                            # held-back chunks fill the normalize/fin(3)
                            # latency at the drain boundary
                            app([op_item(8 + c, ct, c if ct else None)
                                 for c in range(2, 4) for ct in range(2)])
                while queue:
                    drip()
                for i in range(4):
                    op_chunk(12 + i, 0)
                    op_chunk(12 + i, 1)
                    rs_chunk(4 + i)

    nc.compile()
    return nc


_NC_CACHE = None


def _get_nc():
    global _NC_CACHE
    if _NC_CACHE is None:
        _NC_CACHE = build()
    return _NC_CACHE


def _in_maps(x, Wqkv, bqkv, Wo, bo):
    x = np.ascontiguousarray(np.asarray(x, dtype=np.float32))
    Wqkv = np.ascontiguousarray(np.asarray(Wqkv, dtype=np.float32))
    bqkv = np.asarray(bqkv, dtype=np.float32)
    Wo = np.ascontiguousarray(np.asarray(Wo, dtype=np.float32))
    bo = np.asarray(bo, dtype=np.float32)

    from ml_dtypes import bfloat16
    identb = np.eye(128, dtype=bfloat16)
    i_ = np.arange(128, dtype=np.int64)[:, None]
    j_ = np.arange(128, dtype=np.int64)[None, :]
    tri01 = np.where(i_ > j_, np.float32(0.0), np.float32(1.0)).astype(bfloat16)
    consts = np.concatenate([identb, tri01], axis=1)

    # x^T SBUF images: [tt, 128, cc*512] with xtd[tt][p][cc*512+t] =
    # x[tt*512+t, cc*128+p]
    xts = {}
    xb0s = {}
    for b in range(B):
        xt = x[b].T.astype(bfloat16)              # [C, T]
        xt = xt.reshape(NCC, 128, NTT, 512)
        xts[b] = np.ascontiguousarray(
            xt.transpose(2, 1, 0, 3).reshape(NTT, 128, NCC * 512))
        xb0s[b] = np.ascontiguousarray(x[b][:512]).astype(bfloat16)

    def w_img(W, fblk):
        # [C, F] -> [128, (F//fblk)*NCC*fblk] fb-major partition-major image:
        # col ((fb*NCC + cc)*fblk + j) <- W[cc*128 + p, fb*fblk + j]
        F = W.shape[1]
        nfb = F // fblk
        w = W.reshape(NCC, 128, nfb, fblk)          # [cc, p, fb, j]
        return np.ascontiguousarray(
            w.transpose(1, 2, 0, 3).reshape(128, F * NCC)
        ).astype(bfloat16)

    in_maps = []
    for core in range(8):
        b, hh = core // 2, core % 2
        sl = slice(hh * FL, (hh + 1) * FL)
        bv_loc = bqkv[2 * C:][sl]
        wo_loc = np.ascontiguousarray(Wo[sl, :])
        # V bias folded into output bias: attn rows sum to 1 after normalize
        bo_loc = bo * 0.5 + bv_loc @ wo_loc
        # wo image over its 4 cc chunks of 128 (FL=512 rows)
        wo_img = np.ascontiguousarray(
            wo_loc.reshape(NHP, 128, C).transpose(1, 0, 2).reshape(128, NHP * C)
        ).astype(bfloat16)
        in_maps.append({
            "xb0": xb0s[b],
            "xtd": xts[b],
            "wq": w_img(np.ascontiguousarray(Wqkv[:, 0 * C:1 * C][:, sl]), 128),
            "wk": w_img(np.ascontiguousarray(Wqkv[:, 1 * C:2 * C][:, sl]), 128),
            "wv": w_img(np.ascontiguousarray(Wqkv[:, 2 * C:3 * C][:, sl]), 256),
            "wo": wo_img,
            "bqk": np.ascontiguousarray(np.concatenate([
                bqkv[1 * C:2 * C][sl].reshape(NHP, 128).T,
                bqkv[0 * C:1 * C][sl].reshape(NHP, 128).T,
            ], axis=1)),
            "bob": np.broadcast_to(bo_loc[None, :], (128, C)).copy(),
            "consts": consts,
        })

    return in_maps


def _assemble(res):
    out = np.empty((B, T, C), dtype=np.float32)
    for b in range(B):
        out[b, : T // 2] = res.results[2 * b]["zh"]
        out[b, T // 2:] = res.results[2 * b + 1]["zh"]
    return out


def kernel(x, Wqkv, bqkv, Wo, bo):
    in_maps = _in_maps(x, Wqkv, bqkv, Wo, bo)
    res = run_bass_kernel_spmd(_get_nc(), in_maps, core_ids=list(range(8)))
    return _assemble(res)


def run_traced(x, Wqkv, bqkv, Wo, bo, trace_cores=None):
    in_maps = _in_maps(x, Wqkv, bqkv, Wo, bo)
    res = run_bass_kernel_spmd(
        _get_nc(), in_maps, core_ids=list(range(8)), trace=True,
        trace_cores=trace_cores,
    )
    return res


# revision 96
# speedup vs baseline: 1.0031x; 1.0016x over previous
"""Causal multi-head attention block (QKV proj -> causal MHA -> out proj) on 8 Trainium2
cores.

Sharding: core = b*2 + hh handles batch b (of 4) and head-half hh (8 of 16 heads),
computing attention for its heads over the full sequence, then a partial output
projection over its 512 y-channels for all 2048 tokens. A pairwise ReduceScatter
([0,1],[2,3],...) sums the two partials of each batch and leaves each core with its
token-half of the final output.

Host-side prep: x^T is laid out as the exact SBUF image (the device transposes
only tokens 0-255, providing cheap PE work that burns the clock-ramp window
while the big DMAs stream in); weights are pre-rearranged to fb-major
partition-major images so the first-needed slices are contiguous full-rate
DMAs; the V bias is folded into the output bias.

Device structure per core:
  - All work is cut into "pieces" (QKV projection halves, x^T transposes,
    y-transposes, out-projection chunks) that are dripped into the attention
    inner loop, which is otherwise Act-bound (softmax exp chain).
  - The attention pipeline is flat across (qtile, head-pair) units: the next
    unit's first scores are emitted inside the previous unit's last key-group
    iteration, so the Act engine never drains at unit boundaries.
  - Causal masking multiplies the diagonal 128x128 bands of the attention
    weights by a 0/1 triangle on the otherwise-idle GPSIMD engine (SBUF only),
    keeping the scores->exp chain free of extra hops.
  - attnV computes y transposed (queries on PSUM partitions, via a ones column
    in V for the softmax denominator) so normalization is a per-partition
    tensor_scalar multiply; y is transposed back on the PE for the out-proj.

dtypes: bf16 matmul operands everywhere; PSUM accumulation f32; the softmax
denominator is accumulated from the same bf16 weights, so normalization is
exact w.r.t. rounding.
"""

import numpy as np

import concourse.bass as bass
import concourse.tile as tile
from concourse import bacc, mybir
from concourse.bass_utils import run_bass_kernel_spmd

F32 = mybir.dt.float32
BF16 = mybir.dt.bfloat16
AF = mybir.ActivationFunctionType

B, T, C, H = 4, 2048, 1024, 16
D = C // H          # 64
NHL = H // 2        # 8 local heads per core
NHP = NHL // 2      # 4 local head pairs
FL = NHL * D        # 512 local features
NCC = C // 128      # 8 contraction chunks over C
NTB = T // 128      # 16 token blocks
NTT = T // 512      # 4 token tiles / qtiles
NEG = -1.0e30


def build():
    nc = bacc.Bacc("TRN2", target_bir_lowering=False, num_devices=8)

    xb0 = nc.dram_tensor("xb0", [512, C], BF16, kind="ExternalInput")
    xtd = nc.dram_tensor("xtd", [NTT, 128, NCC * 512], BF16, kind="ExternalInput")
    wk = nc.dram_tensor("wk", [128, NCC * FL], BF16, kind="ExternalInput")
    wq = nc.dram_tensor("wq", [128, NCC * FL], BF16, kind="ExternalInput")
    wv = nc.dram_tensor("wv", [128, NCC * FL], BF16, kind="ExternalInput")
    wo = nc.dram_tensor("wo", [128, NHP * C], BF16, kind="ExternalInput")
    bqk = nc.dram_tensor("bqk", [128, 2 * NHP], F32, kind="ExternalInput")
    bob = nc.dram_tensor("bob", [128, C], F32, kind="ExternalInput")  # (bo/2+bv@Wo) bcast
    consts = nc.dram_tensor("consts", [128, 256], BF16, kind="ExternalInput")
    zh = nc.dram_tensor("zh", [T // 2, C], F32, kind="ExternalOutput")

    with tile.TileContext(nc) as tc:
        with (
            tc.tile_pool(name="res", bufs=1) as res,
            tc.tile_pool(name="dram", bufs=1, space="DRAM") as dram,
        ):
            # resident: Q^T, K^T bf16 [128, 4hp x 2048tok]; V+ones bf16
            # [128, 16tb x 520]; 0/1 triangle, identity + out-proj consts.
            qt_sb = res.tile([128, NHP * T], BF16)
            kt_sb = res.tile([128, NHP * T], BF16)
            v_sb = res.tile([128, NTB * (NHL * 65)], BF16)
            con_sb = res.tile([128, 256], BF16, name="con")
            wo_sb = res.tile([128, NHP * C], BF16, name="wo_sb")
            bob_sb = res.tile([128, C], F32, name="bob_sb")
            zpart = dram.tile([T, C], F32)
            zreds = [
                dram.tile([128, C], F32, name=f"zred{i}") for i in range(8)
            ]
            idb_sb = con_sb[:, 0:128]
            tri_sb = con_sb[:, 128:256]

            with (
                tc.tile_pool(name="p1c", bufs=1) as p1c,
                tc.tile_pool(name="p1", bufs=4) as p1,
                tc.tile_pool(name="xtp", bufs=2) as xtp,
                tc.tile_pool(name="ytp", bufs=4) as ytp,
                tc.tile_pool(name="ysb_pool", bufs=1) as ysb_pool,
                tc.tile_pool(name="p2", bufs=10) as p2,
                tc.tile_pool(name="norm", bufs=12) as norm,
                tc.tile_pool(name="p3", bufs=10) as p3,
                tc.tile_pool(name="s_ps", bufs=2, space="PSUM") as s_ps_pool,
                tc.tile_pool(name="yu_ps", bufs=2, space="PSUM") as yu_ps_pool,
                tc.tile_pool(name="scr_ps", bufs=2, space="PSUM") as scr_ps_pool,
            ):
                ysb = ysb_pool.tile([128, NHP * T], BF16)

                # ---- preamble DMAs (order = queue order = first-use order) ----
                wk_sb = p1c.tile([128, NCC * FL], BF16, tag="wk")
                wq_sb = p1c.tile([128, NCC * FL], BF16, tag="wq")
                wv_sb = p1c.tile([128, NCC * FL], BF16, tag="wv")
                bqk_sb = p1c.tile([128, 2 * NHP], F32, tag="bqk")
                bk_sb = bqk_sb[:, 0:NHP]
                bq_sb = bqk_sb[:, NHP:2 * NHP]

                xnats = {}
                for tb in range(2):
                    xnats[tb] = p1.tile([128, C], BF16, tag="xnat",
                                        name=f"xnat{tb}")
                nc.sync.dma_start(xnats[0][:], xb0[0:128, :])
                nc.sync.dma_start(con_sb[:], consts[:, :])
                nc.sync.dma_start(bqk_sb[:], bqk[:, :])
                nc.sync.dma_start(xnats[1][:], xb0[128:256, :])

                def x3(x):
                    return x.rearrange("p (c t) -> p c t", c=NCC)

                xts = {}

                def load_xt(tt):
                    xts[tt] = xtp.tile([128, NCC * 512], BF16, tag="xt",
                                       name=f"xt{tt}")
                    if tt == 0:
                        # tokens 0-255 come from on-device transposes (cheap
                        # PE work that burns the clock-ramp window); the
                        # rest streams from the host-side x^T image
                        nc.sync.dma_start(x3(xts[0])[:, :, 256:512],
                                          x3(xtd[0, :, :])[:, :, 256:512])
                    else:
                        nc.sync.dma_start(xts[tt][:], xtd[tt, :, :])

                # weights are host-arranged fb-major, so the first-needed
                # feature-block slices are contiguous full-rate DMAs
                nc.sync.dma_start(wk_sb[:, 0:1024], wk[:, 0:1024])
                nc.sync.dma_start(wq_sb[:, 0:1024], wq[:, 0:1024])
                nc.sync.dma_start(wv_sb[:, 0:2048], wv[:, 0:2048])
                load_xt(0)
                # warm the exp table (hides ~2.7us ACT_TABLE_LOAD)
                warm = p1c.tile([1, 1], F32, tag="warm")
                nc.scalar.activation(warm[:], bqk_sb[0:1, 0:1], AF.Exp)
                nc.sync.dma_start(wk_sb[:, 1024:4096], wk[:, 1024:4096])
                nc.sync.dma_start(wq_sb[:, 1024:4096], wq[:, 1024:4096])
                nc.sync.dma_start(wv_sb[:, 2048:4096], wv[:, 2048:4096])
                load_xt(1)
                nc.sync.dma_start(wo_sb[:], wo[:, :])
                nc.sync.dma_start(bob_sb[:], bob[:, :])

                # ---- pieces: x^T transposes (tile 0), QKV halves ----
                npieces = {tt: 0 for tt in range(NTT)}
                NP_TT = 24  # qkv pieces per tile (16 qk halves + 8 v halves)

                def piece_done(tt):
                    # last piece of tile tt frees its xt buffer: queue the
                    # DMA for tt+2 right here (xtp bufs=2)
                    npieces[tt] += 1
                    full = NP_TT + (2 if tt == 0 else 0)
                    if npieces[tt] == full and tt + 2 < NTT:
                        load_xt(tt + 2)

                def tp_piece(k):
                    # transpose token block k into x^T tile 0
                    xnat = xnats.pop(k)
                    xt = xts[0]
                    for cg in range(NCC // 4):
                        tp_ps = scr_ps_pool.tile([128, 512], BF16, tag="scr",
                                                 name=f"tp{k}_{cg}")
                        for kk in range(4):
                            cc = cg * 4 + kk
                            nc.tensor.transpose(
                                tp_ps[:, kk * 128:(kk + 1) * 128],
                                xnat[:, cc * 128:(cc + 1) * 128], idb_sb[:]
                            )
                        dst = xt[:].rearrange("p (c t) -> p c t", c=NCC)[
                            :, cg * 4:(cg + 1) * 4, k * 128:(k + 1) * 128
                        ]
                        src = tp_ps[:].rearrange("p (k t) -> p k t", k=4)
                        nc.vector.tensor_scalar_add(dst, src, 0.0)
                    piece_done(0)

                def qk_half(tt, fb, th, w_sb, b_sb, dst):
                    # K or Q projection for feature block fb, token half th
                    xt = xts[tt]
                    ps = scr_ps_pool.tile([128, 256], F32, tag="scr",
                                          name=f"ps{tt}_{fb}_{th}")
                    for cc in range(NCC):
                        nc.tensor.matmul(
                            ps[:],
                            w_sb[:, (fb * NCC + cc) * 128:
                                 (fb * NCC + cc + 1) * 128],
                            xt[:, cc * 512 + th * 256: cc * 512 + (th + 1) * 256],
                            start=(cc == 0),
                            stop=(cc == NCC - 1),
                        )
                    nc.vector.tensor_scalar_add(
                        dst[:, fb * T + tt * 512 + th * 256:
                            fb * T + tt * 512 + (th + 1) * 256],
                        ps[:],
                        b_sb[:, fb:fb + 1],
                    )
                    piece_done(tt)

                def v_half(tt, k, fh):
                    # V projection for token block 4tt+k, feature half fh
                    tb = 4 * tt + k
                    xt = xts[tt]
                    ps = scr_ps_pool.tile([128, 256], F32, tag="scr",
                                          name=f"psv{tb}_{fh}")
                    for cc in range(NCC):
                        nc.tensor.matmul(
                            ps[:],
                            xt[:, cc * 512 + k * 128: cc * 512 + (k + 1) * 128],
                            wv_sb[:, (fh * NCC + cc) * 256:
                                  (fh * NCC + cc + 1) * 256],
                            start=(cc == 0),
                            stop=(cc == NCC - 1),
                        )
                    vslice = v_sb[:, tb * (NHL * 65):(tb + 1) * (NHL * 65)]
                    v3 = vslice.rearrange("p (h c) -> p h c", h=NHL)
                    nc.vector.tensor_scalar_add(
                        v3[:, fh * 4:(fh + 1) * 4, 0:D],
                        ps[:].rearrange("p (h d) -> p h d", h=4),
                        0.0,
                    )
                    nc.gpsimd.memset(v3[:, fh * 4:(fh + 1) * 4, D:D + 1], 1.0)
                    piece_done(tt)

                # ---- out-proj + y-transpose + ReduceScatter pieces ----
                # zpart rows chunk-major: chunk c holds tb c then tb 8+c, so
                # each pairwise ReduceScatter chunk is one contiguous block.
                ZROW = {}
                for c in range(8):
                    ZROW[c] = c * 256
                    ZROW[8 + c] = c * 256 + 128

                y_ts = {}

                def fin_piece(qt, tl):
                    # transpose y_t -> ysb feature-major for the out-proj
                    y_t = y_ts[qt]
                    tb = 4 * qt + tl
                    tp = scr_ps_pool.tile([128, 512], BF16, tag="scr",
                                          name=f"ytp{qt}_{tl}")
                    for fc in range(NHP):
                        nc.tensor.transpose(
                            tp[:, fc * 128:(fc + 1) * 128],
                            y_t[:, tl * 512 + fc * 128:
                                tl * 512 + (fc + 1) * 128],
                            idb_sb[:],
                        )
                    nc.vector.tensor_scalar_add(
                        ysb[:].rearrange("p (h t) -> p h t", h=NHP)[
                            :, :, tb * 128:(tb + 1) * 128],
                        tp[:].rearrange("p (h t) -> p h t", h=NHP),
                        0.0,
                    )

                def op_chunk(tb, ct):
                    zrow = ZROW[tb]
                    zps = scr_ps_pool.tile([128, 512], F32, tag="scr",
                                           name=f"z{tb}_{ct}")
                    for cc in range(NHP):
                        nc.tensor.matmul(
                            zps[:],
                            ysb[:, cc * T + tb * 128: cc * T + (tb + 1) * 128],
                            wo_sb[:, cc * C + ct * 512: cc * C + (ct + 1) * 512],
                            start=(cc == 0),
                            stop=(cc == NHP - 1),
                        )
                    z_sb = p3.tile(
                        [128, 512], F32, tag="zsb", name=f"zsb{tb}_{ct}"
                    )
                    nc.vector.tensor_add(
                        z_sb[:], zps[:], bob_sb[:, ct * 512:(ct + 1) * 512]
                    )
                    nc.sync.dma_start(
                        zpart[zrow:zrow + 128, ct * 512:(ct + 1) * 512],
                        z_sb[:],
                    )

                def rs_chunk(c):
                    # (the compiler rejects collectives writing IO tensors,
                    # so stage through zreds and DMA into zh)
                    nc.gpsimd.collective_compute(
                        "ReduceScatter",
                        mybir.AluOpType.add,
                        replica_groups=[[0, 1], [2, 3], [4, 5], [6, 7]],
                        ins=[zpart[c * 256:(c + 1) * 256, :].opt()],
                        outs=[zreds[c].opt()],
                    )
                    nc.sync.dma_start(
                        zh[c * 128:(c + 1) * 128, :], zreds[c][:]
                    )

                # ---- piece queue: dripped into attention units ----
                queue = []
                emitted = [0]

                def drip():
                    if queue:
                        emitted[0] += 1
                        queue.pop(0)()

                def item(fn, *args):
                    def go():
                        fn(*args)
                    return go

                def op_item(tb, ct, c=None):
                    def go():
                        op_chunk(tb, ct)
                        if c is not None:
                            rs_chunk(c)
                    return go

                # queue pieces in dependency order, recording for each
                # attention unit (qt,hp) how many pieces must be emitted
                # before it (its K/Q columns, its V feature half, and for
                # tt0 the x^T transposes).
                prereq = {}
                total = [0]

                def app(ps):
                    queue.extend(ps)
                    total[0] += len(ps)

                def app_qkv(tt):
                    def qk2(fb, w_sb, b_sb, dst):
                        return [item(qk_half, tt, fb, th, w_sb, b_sb, dst)
                                for th in range(2)]

                    if tt == 0:
                        # startup: token-half-major order matching the DMA
                        # stream (tp transposes give tokens 0-255; the
                        # token 256-511 x^T slice lands after wk/wq/wv)
                        app([item(tp_piece, k) for k in range(2)])
                        app([item(qk_half, 0, 0, 0, wk_sb, bk_sb, kt_sb),
                             item(qk_half, 0, 0, 0, wq_sb, bq_sb, qt_sb),
                             item(v_half, 0, 0, 0),
                             item(v_half, 0, 1, 0),
                             item(qk_half, 0, 0, 1, wk_sb, bk_sb, kt_sb),
                             item(qk_half, 0, 0, 1, wq_sb, bq_sb, qt_sb)])
                    else:
                        app(qk2(0, wk_sb, bk_sb, kt_sb)
                            + qk2(0, wq_sb, bq_sb, qt_sb))
                        app([item(v_half, tt, k, 0) for k in range(2)])
                    prereq[(tt, 0)] = total[0]
                    # V k2/k3 are only read by the unit's kg>=1 attnV; the
                    # kg0 drips deliver them, shortening the serial prologue
                    app([item(v_half, tt, k, 0) for k in range(2, 4)])
                    app(qk2(1, wk_sb, bk_sb, kt_sb) + qk2(1, wq_sb, bq_sb, qt_sb))
                    prereq[(tt, 1)] = total[0]
                    app(qk2(2, wk_sb, bk_sb, kt_sb) + qk2(2, wq_sb, bq_sb, qt_sb))
                    app([item(v_half, tt, k, 1) for k in range(4)])
                    prereq[(tt, 2)] = total[0]
                    app(qk2(3, wk_sb, bk_sb, kt_sb) + qk2(3, wq_sb, bq_sb, qt_sb))
                    prereq[(tt, 3)] = total[0]

                for tt in range(NTT):
                    app_qkv(tt)

                # ---- flat cross-unit-pipelined attention ----
                seq = [(qt, hp) for qt in range(NTT) for hp in range(NHP)]
                ustate = {}

                def unit_alloc(ui):
                    qt, hp = seq[ui]
                    if qt not in y_ts:
                        y_ts[qt] = ytp.tile([128, 4 * 512], BF16, tag="yt",
                                            name=f"yt{qt}")
                    ustate[ui] = {
                        "yus": [
                            yu_ps_pool.tile([128, 4 * 65], F32, tag="yu",
                                            name=f"yu{ui}_{i}")
                            for i in range(2)
                        ],
                        "qsl": qt_sb[:, hp * T + qt * 512:
                                     hp * T + (qt + 1) * 512],
                        "sss": {},
                    }

                def emit_scores(ui, kg, hi=None):
                    qt, hp = seq[ui]
                    st = ustate[ui]
                    if hi is None:
                        st["sss"][kg] = [
                            s_ps_pool.tile([128, 1024], F32, tag="s",
                                           name=f"s{ui}_{kg}_{i}")
                            for i in range(2)
                        ]
                        for h2 in range(2):
                            emit_scores(ui, kg, h2)
                        return
                    ss = st["sss"][kg]
                    for c2 in range(2):
                        kb = kg * 2 + c2
                        c = kb - 4 * qt
                        # c==1 writes the full block so the untrimmed exp
                        # never reads unwritten PSUM (extra cols unused)
                        j0 = c * 128 if c >= 2 else 0
                        nc.tensor.matmul(
                            ss[hi][:, c2 * 512 + j0:(c2 + 1) * 512],
                            kt_sb[
                                hi * 64:(hi + 1) * 64,
                                hp * T + kb * 128: hp * T + (kb + 1) * 128,
                            ],
                            st["qsl"][hi * 64:(hi + 1) * 64, j0:],
                            tile_position=(hi * 64, 0),
                            start=True,
                            stop=True,
                        )


                def emit_exp(ui, kg, hi, at):
                    qt, hp = seq[ui]
                    ss = ustate[ui]["sss"][kg]
                    j0r = []
                    for c2 in range(2):
                        c = kg * 2 + c2 - 4 * qt
                        j0r.append(c * 128 if c > 0 else 0)
                    if j0r[0] >= 256:
                        # heavily masked pair: exp valid suffixes only
                        nc.scalar.activation(
                            at[:, j0r[0]:512], ss[hi][:, j0r[0]:512],
                            AF.Exp, scale=0.125,
                        )
                        nc.scalar.activation(
                            at[:, 512 + j0r[1]:1024],
                            ss[hi][:, 512 + j0r[1]:1024],
                            AF.Exp, scale=0.125,
                        )
                    else:
                        nc.scalar.activation(
                            at[:], ss[hi][:], AF.Exp, scale=0.125
                        )
                    # zero the upper triangle of diagonal bands (gpsimd,
                    # SBUF-only) instead of adding -inf before the exp:
                    # keeps the scores->exp chain short and the DVE free.
                    for c2 in range(2):
                        c = kg * 2 + c2 - 4 * qt
                        if 0 <= c <= 3:
                            b0 = c2 * 512 + c * 128
                            nc.gpsimd.tensor_mul(
                                at[:, b0:b0 + 128],
                                at[:, b0:b0 + 128],
                                tri_sb[:],
                            )

                def emit_attnv(ui, kg, hi, at):
                    # Emits the unmasked attnV blocks; returns a closure for
                    # the masked diagonal bands, deferred until after both
                    # his' main work so the serial gpsimd mask multiplies
                    # overlap fat PE work instead of gating it.
                    qt, hp = seq[ui]
                    st = ustate[ui]
                    n_kb = 4 * (qt + 1)
                    h = 2 * hp + hi

                    def mm(c2, kb, qoff):
                        vsl = v_sb[
                            :,
                            kb * (NHL * 65) + h * 65:
                            kb * (NHL * 65) + h * 65 + 65,
                        ]
                        nc.tensor.matmul(
                            st["yus"][hi][:, qoff * 65:(qoff + 1) * 65],
                            at[:, c2 * 512 + qoff * 128:
                               c2 * 512 + (qoff + 1) * 128],
                            vsl,
                            start=(kb == 0 and qoff == 3),
                            stop=(kb == n_kb - 1),
                        )

                    bands = []
                    for c2 in range(2):
                        kb = kg * 2 + c2
                        # qoff descending: the very first emitted write of
                        # the unit carries start=True (clears the yu bank)
                        qlo = max(kb - 4 * qt, 0)
                        for qoff in range(3, qlo - 1, -1):
                            if qoff == kb - 4 * qt:
                                bands.append((c2, kb, qoff))
                            else:
                                mm(c2, kb, qoff)

                    def emit_bands():
                        for c2, kb, qoff in bands:
                            mm(c2, kb, qoff)
                    return emit_bands

                def normalize_hi(ui, hi):
                    # emitted right after this hi's last attnV so the yu
                    # buffer frees (and y_t fills) as early as possible.
                    # One DVE multiply covers all four qoff blocks via a
                    # stride-0 broadcast of the per-(partition,qoff)
                    # reciprocal over the 64 head-dim columns.
                    qt, hp = seq[ui]
                    st = ustate[ui]
                    y_t = y_ts[qt]
                    yu3 = st["yus"][hi][:].rearrange("p (b c) -> p b c", b=4)
                    rcp = norm.tile([128, 4], F32, tag="rcp")
                    nc.vector.reciprocal(
                        rcp[:].unsqueeze(2), yu3[:, :, 64:65]
                    )
                    h = 2 * hp + hi
                    dst = y_t[:].rearrange("p (q f) -> p q f", q=4)[
                        :, :, h * 64:(h + 1) * 64]
                    nc.vector.tensor_mul(
                        dst, yu3[:, :, 0:64],
                        rcp[:].unsqueeze(2).broadcast_to((128, 4, 64)),
                    )

                def drain_to(n):
                    while emitted[0] < n and queue:
                        drip()

                for ui, (qt, hp) in enumerate(seq):
                    n_kg = 2 * (qt + 1)
                    drain_to(prereq[(qt, hp)])
                    if ui == 0:
                        unit_alloc(0)
                        emit_scores(0, 0)
                    for kg in range(n_kg):
                        last_kg = kg == n_kg - 1
                        if last_kg and ui + 1 < len(seq):
                            drain_to(prereq[seq[ui + 1]])
                        # qt>=2 windows must not drain the queue early:
                        # their per-kg piece deficit is ~1 piece, so drip
                        # once per kg there and twice per kg before.
                        drips = (2 if qt < 2 else 1)
                        bandfns = []
                        for hi in range(2):
                            at = p2.tile([128, 1024], BF16, tag="attn")
                            emit_exp(ui, kg, hi, at)
                            if not last_kg:
                                if hi == 0:
                                    ustate[ui]["sss"][kg + 1] = [
                                        s_ps_pool.tile(
                                            [128, 1024], F32, tag="s",
                                            name=f"s{ui}_{kg + 1}_{i}")
                                        for i in range(2)
                                    ]
                                emit_scores(ui, kg + 1, hi)
                            elif ui + 1 < len(seq):
                                if hi == 0:
                                    unit_alloc(ui + 1)
                                    ustate[ui + 1]["sss"][0] = [
                                        s_ps_pool.tile(
                                            [128, 1024], F32, tag="s",
                                            name=f"s{ui + 1}_0_{i}")
                                        for i in range(2)
                                    ]
                                emit_scores(ui + 1, 0, hi)
                            # drip BEFORE the attnV batch: the ~12 attnV
                            # matmuls all wait on the exp semaphore, and the
                            # PE wait-queue is only 4 deep — emitting them
                            # first would block the sequencer before the
                            # ready fill work behind them could dispatch
                            if hi < drips:
                                drip()
                            bandfns.append(emit_attnv(ui, kg, hi, at))
                        for fn in bandfns:
                            fn()
                        if last_kg:
                            normalize_hi(ui, 0)
                            normalize_hi(ui, 1)
                    if hp == NHP - 1:
                        # unit-set (qtile) complete: queue its y-transposes
                        # and, once qt>=1 transposes exist, the out-proj
                        # chunks that only need earlier qtiles.
                        app([item(fin_piece, qt, tl) for tl in range(4)])
                        if qt == 2:
                            app([op_item(tb, ct)
                                 for tb in range(8) for ct in range(2)])
                            app([op_item(8 + c, ct, c if ct else None)
                                 for c in range(2) for ct in range(2)])
                        if qt == 3:
                            # held-back chunks fill the normalize/fin(3)
                            # latency at the drain boundary
                            app([op_item(8 + c, ct, c if ct else None)
                                 for c in range(2, 4) for ct in range(2)])
                while queue:
                    drip()
                for i in range(4):
                    op_chunk(12 + i, 0)
                    op_chunk(12 + i, 1)
                    rs_chunk(4 + i)

    nc.compile()
    return nc


_NC_CACHE = None


def _get_nc():
    global _NC_CACHE
    if _NC_CACHE is None:
        _NC_CACHE = build()
    return _NC_CACHE


def _in_maps(x, Wqkv, bqkv, Wo, bo):
    x = np.ascontiguousarray(np.asarray(x, dtype=np.float32))
    Wqkv = np.ascontiguousarray(np.asarray(Wqkv, dtype=np.float32))
    bqkv = np.asarray(bqkv, dtype=np.float32)
    Wo = np.ascontiguousarray(np.asarray(Wo, dtype=np.float32))
    bo = np.asarray(bo, dtype=np.float32)

    from ml_dtypes import bfloat16
    identb = np.eye(128, dtype=bfloat16)
    i_ = np.arange(128, dtype=np.int64)[:, None]
    j_ = np.arange(128, dtype=np.int64)[None, :]
    tri01 = np.where(i_ > j_, np.float32(0.0), np.float32(1.0)).astype(bfloat16)
    consts = np.concatenate([identb, tri01], axis=1)

    # x^T SBUF images: [tt, 128, cc*512] with xtd[tt][p][cc*512+t] =
    # x[tt*512+t, cc*128+p]
    xts = {}
    xb0s = {}
    for b in range(B):
        xt = x[b].T.astype(bfloat16)              # [C, T]
        xt = xt.reshape(NCC, 128, NTT, 512)
        xts[b] = np.ascontiguousarray(
            xt.transpose(2, 1, 0, 3).reshape(NTT, 128, NCC * 512))
        xb0s[b] = np.ascontiguousarray(x[b][:512]).astype(bfloat16)

    def w_img(W, fblk):
        # [C, F] -> [128, (F//fblk)*NCC*fblk] fb-major partition-major image:
        # col ((fb*NCC + cc)*fblk + j) <- W[cc*128 + p, fb*fblk + j]
        F = W.shape[1]
        nfb = F // fblk
        w = W.reshape(NCC, 128, nfb, fblk)          # [cc, p, fb, j]
        return np.ascontiguousarray(
            w.transpose(1, 2, 0, 3).reshape(128, F * NCC)
        ).astype(bfloat16)

    in_maps = []
    for core in range(8):
        b, hh = core // 2, core % 2
        sl = slice(hh * FL, (hh + 1) * FL)
        bv_loc = bqkv[2 * C:][sl]
        wo_loc = np.ascontiguousarray(Wo[sl, :])
        # V bias folded into output bias: attn rows sum to 1 after normalize
        bo_loc = bo * 0.5 + bv_loc @ wo_loc
        # wo image over its 4 cc chunks of 128 (FL=512 rows)
        wo_img = np.ascontiguousarray(
            wo_loc.reshape(NHP, 128, C).transpose(1, 0, 2).reshape(128, NHP * C)
        ).astype(bfloat16)
        in_maps.append({
            "xb0": xb0s[b],
            "xtd": xts[b],
            "wq": w_img(np.ascontiguousarray(Wqkv[:, 0 * C:1 * C][:, sl]), 128),
            "wk": w_img(np.ascontiguousarray(Wqkv[:, 1 * C:2 * C][:, sl]), 128),
            "wv": w_img(np.ascontiguousarray(Wqkv[:, 2 * C:3 * C][:, sl]), 256),
            "wo": wo_img,
            "bqk": np.ascontiguousarray(np.concatenate([
                bqkv[1 * C:2 * C][sl].reshape(NHP, 128).T,
                bqkv[0 * C:1 * C][sl].reshape(NHP, 128).T,
            ], axis=1)),
            "bob": np.broadcast_to(bo_loc[None, :], (128, C)).copy(),
            "consts": consts,
        })

    return in_maps


def _assemble(res):
    out = np.empty((B, T, C), dtype=np.float32)
    for b in range(B):
        out[b, : T // 2] = res.results[2 * b]["zh"]
        out[b, T // 2:] = res.results[2 * b + 1]["zh"]
    return out


def kernel(x, Wqkv, bqkv, Wo, bo):
    in_maps = _in_maps(x, Wqkv, bqkv, Wo, bo)
    res = run_bass_kernel_spmd(_get_nc(), in_maps, core_ids=list(range(8)))
    return _assemble(res)


def run_traced(x, Wqkv, bqkv, Wo, bo, trace_cores=None):
    in_maps = _in_maps(x, Wqkv, bqkv, Wo, bo)
    res = run_bass_kernel_spmd(
        _get_nc(), in_maps, core_ids=list(range(8)), trace=True,
        trace_cores=trace_cores,
    )
    return res
